# revision 24
# baseline (speedup 1.0000x reference)
"""Multi-head attention (B=8, N=1024, C=1024, H=16) on 8 TRN2 NeuronCores.

Strategy: pure data parallelism — each core computes one batch element with
replicated weights (no collectives). All matmul operands are bf16 (fast
weight loads + half DMA); PSUM accumulation stays fp32.

The kernel is emitted as ONE software-pipelined stream so the scalar engine
(the only engine with exp) is busy wall-to-wall instead of only during a
trailing attention phase:

  per head-pair hp (2 heads), per query-half ic, per key-block kb:
    QK: S^T[key128, q512] for both heads via row-group-paired matmuls
    ACT: P = exp(S*scale)  (scalar engine — the critical resource)
    PV: O^T[65, q512] += V_hat^T @ P^T   (row 64 = softmax sums via ones col)
  "filler" work (QKV projections for later pairs, V projection, first half
  of the output projection) is interleaved into the tensor engine's slack.

Layouts (host-prepped, partition-major so every matmul contracts on
partitions): x/wv/wo as [128, 8*1024] swizzled blocks, wqk packed per
head-pair [128, 8*(128k|128q)]. Output is outT [C, N] fp32, host transposes.
"""
import numpy as np

B, N, C = 8, 1024, 1024
H = 16
HD = C // H               # 64
SCALE = HD ** (-0.5)
NCORES = 8
CB = 8                    # contraction blocks (C / 128)
TB = 8                    # token blocks (N / 128)
VW = 65                   # per-head V width (64 feats + ones col)

_COMPILED = {}


def _build():
    import concourse.bass as bass
    import concourse.tile as tile
    from concourse import bacc, mybir

    F32 = mybir.dt.float32
    BF16 = mybir.dt.bfloat16
    EXP = mybir.ActivationFunctionType.Exp
    ADD = mybir.AluOpType.add

    nc = bacc.Bacc("TRN2", target_bir_lowering=False, debug=False)

    x_d = nc.dram_tensor("x_d", [128, CB * N], BF16, kind="ExternalInput").ap()
    x8_d = nc.dram_tensor("x8_d", [128, CB * N], mybir.dt.float8e4,
                          kind="ExternalInput").ap()
    wqk8_d = nc.dram_tensor("wqk8_d", [8 * 128, CB * 128], mybir.dt.float8e4,
                            kind="ExternalInput").ap()
    wqkq_d = nc.dram_tensor("wqkq_d", [8 * 128, CB * 128], BF16,
                            kind="ExternalInput").ap()
    wv_d = [nc.dram_tensor(f"wv{vc}_d", [128, CB * 512], BF16,
                           kind="ExternalInput").ap() for vc in range(2)]
    wo_d = nc.dram_tensor("wo_d", [128, CB * C], BF16, kind="ExternalInput").ap()
    bqk_d = nc.dram_tensor("bqk_d", [128, 16], F32, kind="ExternalInput").ap()
    bv_d = nc.dram_tensor("bv_d", [1, C], F32, kind="ExternalInput").ap()
    bo_d = nc.dram_tensor("bo_d", [128, 8], F32, kind="ExternalInput").ap()
    outT = nc.dram_tensor("outT", [C, N], F32, kind="ExternalOutput").ap()

    with nc.allow_low_precision(reason="attention: 2e-2 tolerance, bf16 "
                                "normalization chain is well within budget"), \
         tile.TileContext(nc) as tc:
        with tc.tile_pool(name="misc", bufs=1) as pool_misc, \
             tc.tile_pool(name="w", bufs=1) as pool_w, \
             tc.tile_pool(name="qk", bufs=1) as pool_qk, \
             tc.tile_pool(name="V", bufs=1) as pool_V, \
             tc.tile_pool(name="pt", bufs=3) as pool_pt, \
             tc.tile_pool(name="ocp", bufs=1) as pool_ocp, \
             tc.tile_pool(name="opart", bufs=1) as pool_opart, \
             tc.tile_pool(name="norm", bufs=2) as pool_norm, \
             tc.tile_pool(name="outp", bufs=2) as pool_out, \
             tc.tile_pool(name="ps_S", bufs=2, space="PSUM") as ps_S, \
             tc.tile_pool(name="ps_O", bufs=1, space="PSUM") as ps_O, \
             tc.tile_pool(name="ps_proj", bufs=2, space="PSUM") as ps_proj:

            # ---- small constants ----
            bqk_sb = pool_misc.tile([128, 16], F32, tag="bqk")
            bv_sb = pool_misc.tile([1, C], F32, tag="bv")
            bo_sb = pool_misc.tile([128, 8], F32, tag="bo")
            nc.sync.dma_start(bqk_sb[:, :], bqk_d)
            nc.sync.dma_start(bv_sb[:, :], bv_d)
            nc.sync.dma_start(bo_sb[:, :], bo_d)
            bv_rep = pool_misc.tile([128, C], F32, tag="bvrep")
            nc.gpsimd.partition_broadcast(bv_rep[:, :], bv_sb[0:1, :])

            # ---- big SBUF tensors ----
            x_sb = pool_w.tile([128, CB * N], BF16, tag="x")
            F8 = mybir.dt.float8e4
            x8_sb = pool_w.tile([128, CB * N], F8, tag="x8")
            wqk8_sb = [pool_w.tile([128, CB * 128], F8, tag=f"wqk8{hp}", name=f"wqk8{hp}")
                       for hp in range(8)]
            wqkq_sb = [pool_w.tile([128, CB * 128], BF16, tag=f"wqkq{hp}", name=f"wqkq{hp}")
                       for hp in range(8)]
            wv_sb = [pool_w.tile([128, CB * 512], BF16, tag=f"wv{vc}", name=f"wv{vc}")
                     for vc in range(2)]
            wo_sb = pool_w.tile([128, CB * C], BF16, tag="wo")
            qkk_sb = [pool_qk.tile([128, N], BF16, tag=f"qkk{hp}", name=f"qkk{hp}") for hp in range(8)]
            qkq_sb = [pool_qk.tile([128, N], BF16, tag=f"qkq{hp}", name=f"qkq{hp}") for hp in range(8)]
            A_sb = qkq_sb  # normalized attention outputs reuse the q tiles
            V_sb = [pool_V.tile([128, H * VW], BF16, tag=f"V{tb}", name=f"V{tb}") for tb in range(TB)]
            o_part = [pool_opart.tile([128, 512], BF16, tag=f"op{g}", name=f"op{g}")
                      for g in range(16)]

            # ---- PE warm-up: ~9us of dummy matmuls so the HAM clock gate
            # opens during the input DMA and projections start at 2.4 GHz
            warm_sb = pool_misc.tile([128, 512], BF16, tag="warm")
            nc.vector.memset(warm_sb[:, :], 0.0)
            for w in range(40):
                ps = ps_proj.tile([128, 512], F32, tag="pp", name="pp")
                nc.tensor.matmul(ps[:, :], warm_sb[:, 0:128], warm_sb[:, :],
                                 start=True, stop=True)

            # ---- input DMA: x/wv/wo on sync, wqkp on scalar (parallel
            # queues; ACT is idle early). V ones columns via memset.
            for tb in range(TB):
                nc.vector.memset(V_sb[tb][:, 64::VW], 1.0)
            nc.scalar.dma_start(x8_sb[:, :], x8_d)
            nc.sync.dma_start(x_sb[:, :], x_d)
            nc.scalar.dma_start(wqk8_sb[0][:, :], wqk8_d[0:128, :])
            nc.scalar.dma_start(wqkq_sb[0][:, :], wqkq_d[0:128, :])
            for hp in range(1, 8):
                nc.scalar.dma_start(wqk8_sb[hp][:, :],
                                    wqk8_d[hp * 128:(hp + 1) * 128, :])
                nc.scalar.dma_start(wqkq_sb[hp][:, :],
                                    wqkq_d[hp * 128:(hp + 1) * 128, :])
            for vc in range(2):
                nc.sync.dma_start(wv_sb[vc][:, :], wv_d[vc])
            nc.sync.dma_start(wo_sb[:, :], wo_d)

            # ================= emission helpers =================
            MULT = mybir.AluOpType.mult
            DR = mybir.MatmulPerfMode.DoubleRow

            def emit_B_chunk(hp, kq, nch, half, state):
                # kq=0: k features via fp8 DoubleRow (256-row contractions);
                # kq=1: q features via bf16 (accuracy headroom is tighter on q+k
                # jointly, so only one side is quantized to fp8)
                if half == 0:
                    state["ps"] = ps_proj.tile([128, 512], F32, tag="pp", name="pp")
                ps = state["ps"]
                if kq == 0:
                    for t in (2 * half, 2 * half + 1):
                        lhsT = wqk8_sb[hp][:, t * 256:(t + 1) * 256].rearrange(
                            "p (j m) -> p j m", j=2)
                        rhs = x8_sb[:, 2 * t * N:2 * (t + 1) * N].rearrange(
                            "p (j n) -> p j n", j=2)[:, :, nch * 512:(nch + 1) * 512]
                        nc.tensor.matmul(ps[:, :], lhsT, rhs,
                                         start=(t == 0), stop=(t == 3), perf_mode=DR)
                    if half == 1:
                        nc.vector.tensor_scalar(
                            qkk_sb[hp][:, nch * 512:(nch + 1) * 512], ps[:, :],
                            1.0 / 64, bqk_sb[:, 8 + hp:9 + hp], MULT, ADD)
                else:
                    for cb in range(4 * half, 4 * half + 4):
                        nc.tensor.matmul(
                            ps[:, :],
                            wqkq_sb[hp][:, cb * 128:(cb + 1) * 128],
                            x_sb[:, cb * N + nch * 512: cb * N + nch * 512 + 512],
                            start=(cb == 0), stop=(cb == CB - 1),
                        )
                    if half == 1:
                        nc.vector.tensor_scalar(
                            qkq_sb[hp][:, nch * 512:(nch + 1) * 512], ps[:, :],
                            bqk_sb[:, hp:hp + 1], None, ADD)

            def emit_A_chunk(tb, vc, half, state):
                if half == 0:
                    state["ps"] = ps_proj.tile([128, 512], F32, tag="pp", name="pp")
                ps = state["ps"]
                for cb in range(4 * half, 4 * half + 4):
                    nc.tensor.matmul(
                        ps[:, :],
                        x_sb[:, cb * N + tb * 128: cb * N + tb * 128 + 128],
                        wv_sb[vc][:, cb * 512:(cb + 1) * 512],
                        start=(cb == 0), stop=(cb == CB - 1),
                    )
                if half == 1:
                    dst = V_sb[tb][:, vc * 8 * VW:(vc + 1) * 8 * VW]
                    dst3 = dst.rearrange("p (h d) -> p h d", h=8)[:, :, 0:64]
                    src3 = ps[:, :].rearrange("p (h d) -> p h d", h=8)
                    bv3 = bv_rep[:, vc * 512:(vc + 1) * 512].rearrange(
                        "p (h d) -> p h d", h=8)
                    nc.vector.tensor_add(dst3, src3, bv3)

            def emit_D1_chunk(cb, nch, state):
                ps = ps_proj.tile([128, 512], F32, tag="pp", name="pp")
                for hb in range(4):
                    nc.tensor.matmul(
                        ps[:, :],
                        wo_sb[:, hb * C + cb * 128: hb * C + cb * 128 + 128],
                        A_sb[hb][:, nch * 512:(nch + 1) * 512],
                        start=(hb == 0), stop=(hb == 3),
                    )
                nc.vector.tensor_scalar(
                    o_part[cb * 2 + nch][:, :], ps[:, :], bo_sb[:, cb:cb + 1],
                    None, ADD)

            def emit_D1b_chunk(cb, nch):
                # hb 4-5 partials; result = o_part(bias+hb0-3) + these, stored
                # into dead x_sb space so the tail only needs hb 6-7
                ps = ps_proj.tile([128, 512], F32, tag="pp", name="pp")
                for hb in range(4, 6):
                    nc.tensor.matmul(
                        ps[:, :],
                        wo_sb[:, hb * C + cb * 128: hb * C + cb * 128 + 128],
                        A_sb[hb][:, nch * 512:(nch + 1) * 512],
                        start=(hb == 4), stop=(hb == 5),
                    )
                g = cb * 2 + nch
                nc.vector.tensor_add(
                    x_sb[:, g * 512:(g + 1) * 512], ps[:, :], o_part[g][:, :])

            fill_q = []

            def add_B_group(hp, kq, nch):
                st = {}
                if kq == 0:
                    def mk(t):
                        def emit():
                            if t == 0:
                                st["ps"] = ps_proj.tile([128, 512], F32,
                                                        tag="pp", name="pp")
                            ps = st["ps"]
                            lhsT = wqk8_sb[hp][:, t * 256:(t + 1) * 256].rearrange(
                                "p (j m) -> p j m", j=2)
                            rhs = x8_sb[:, 2 * t * N:2 * (t + 1) * N].rearrange(
                                "p (j n) -> p j n", j=2)[:, :, nch * 512:(nch + 1) * 512]
                            nc.tensor.matmul(ps[:, :], lhsT, rhs,
                                             start=(t == 0), stop=(t == 3),
                                             perf_mode=DR)
                            if t == 3:
                                nc.vector.tensor_scalar(
                                    qkk_sb[hp][:, nch * 512:(nch + 1) * 512],
                                    ps[:, :], 1.0 / 64,
                                    bqk_sb[:, 8 + hp:9 + hp], MULT, ADD)
                        return emit
                    for t in range(4):
                        fill_q.append(mk(t))
                else:
                    def mk(cb):
                        def emit():
                            if cb == 0:
                                st["ps"] = ps_proj.tile([128, 512], F32,
                                                        tag="pp", name="pp")
                            ps = st["ps"]
                            nc.tensor.matmul(
                                ps[:, :],
                                wqkq_sb[hp][:, cb * 128:(cb + 1) * 128],
                                x_sb[:, cb * N + nch * 512: cb * N + nch * 512 + 512],
                                start=(cb == 0), stop=(cb == CB - 1),
                            )
                            if cb == CB - 1:
                                nc.vector.tensor_scalar(
                                    qkq_sb[hp][:, nch * 512:(nch + 1) * 512],
                                    ps[:, :], bqk_sb[:, hp:hp + 1], None, ADD)
                        return emit
                    for cb in range(CB):
                        fill_q.append(mk(cb))

            def add_A_group(tb, vc):
                st = {}
                def mk(cb):
                    def emit():
                        if cb == 0:
                            st["ps"] = ps_proj.tile([128, 512], F32,
                                                    tag="pp", name="pp")
                        ps = st["ps"]
                        nc.tensor.matmul(
                            ps[:, :],
                            x_sb[:, cb * N + tb * 128: cb * N + tb * 128 + 128],
                            wv_sb[vc][:, cb * 512:(cb + 1) * 512],
                            start=(cb == 0), stop=(cb == CB - 1),
                        )
                        if cb == CB - 1:
                            dst = V_sb[tb][:, vc * 8 * VW:(vc + 1) * 8 * VW]
                            dst3 = dst.rearrange("p (h d) -> p h d", h=8)[:, :, 0:64]
                            src3 = ps[:, :].rearrange("p (h d) -> p h d", h=8)
                            bv3 = bv_rep[:, vc * 512:(vc + 1) * 512].rearrange(
                                "p (h d) -> p h d", h=8)
                            nc.vector.tensor_add(dst3, src3, bv3)
                    return emit
                for cb in range(CB):
                    fill_q.append(mk(cb))

            def add_D1_group(cb, nch):
                st = {}
                def mk(hb):
                    def emit():
                        if hb == 0:
                            st["ps"] = ps_proj.tile([128, 512], F32,
                                                    tag="pp", name="pp")
                        ps = st["ps"]
                        nc.tensor.matmul(
                            ps[:, :],
                            wo_sb[:, hb * C + cb * 128: hb * C + cb * 128 + 128],
                            A_sb[hb][:, nch * 512:(nch + 1) * 512],
                            start=(hb == 0), stop=(hb == 3),
                        )
                        if hb == 3:
                            nc.vector.tensor_scalar(
                                o_part[cb * 2 + nch][:, :], ps[:, :],
                                bo_sb[:, cb:cb + 1], None, ADD)
                    return emit
                for hb in range(4):
                    fill_q.append(mk(hb))

            for hp in (1, 2, 3, 4):
                for kq in range(2):
                    for nch in range(2):
                        add_B_group(hp, kq, nch)
            for tb in range(TB):
                add_A_group(tb, 1)
            for hp in (5, 6, 7):
                for kq in range(2):
                    for nch in range(2):
                        add_B_group(hp, kq, nch)
            for cb in range(CB):
                for nch in range(2):
                    add_D1_group(cb, nch)

            fill_i = 0

            def drain1(k=1):
                nonlocal fill_i
                for _ in range(k):
                    if fill_i < len(fill_q):
                        fill_q[fill_i]()
                        fill_i += 1

            # ---- pair 0 prologue: its qk projection + first V blocks ----
            def emit_A_direct(tb, vc):
                ps = ps_proj.tile([128, 512], F32, tag="pp", name="pp")
                for cb in range(CB):
                    nc.tensor.matmul(
                        ps[:, :],
                        x_sb[:, cb * N + tb * 128: cb * N + tb * 128 + 128],
                        wv_sb[vc][:, cb * 512:(cb + 1) * 512],
                        start=(cb == 0), stop=(cb == CB - 1),
                    )
                dst = V_sb[tb][:, vc * 8 * VW:(vc + 1) * 8 * VW]
                dst3 = dst.rearrange("p (h d) -> p h d", h=8)[:, :, 0:64]
                src3 = ps[:, :].rearrange("p (h d) -> p h d", h=8)
                bv3 = bv_rep[:, vc * 512:(vc + 1) * 512].rearrange(
                    "p (h d) -> p h d", h=8)
                nc.vector.tensor_add(dst3, src3, bv3)

            def emit_B_direct(kq, nch):
                ps = ps_proj.tile([128, 512], F32, tag="pp", name="pp")
                if kq == 0:
                    for t in range(4):
                        lhsT = wqk8_sb[0][:, t * 256:(t + 1) * 256].rearrange(
                            "p (j m) -> p j m", j=2)
                        rhs = x8_sb[:, 2 * t * N:2 * (t + 1) * N].rearrange(
                            "p (j n) -> p j n", j=2)[:, :, nch * 512:(nch + 1) * 512]
                        nc.tensor.matmul(ps[:, :], lhsT, rhs,
                                         start=(t == 0), stop=(t == 3),
                                         perf_mode=DR)
                    nc.vector.tensor_scalar(
                        qkk_sb[0][:, nch * 512:(nch + 1) * 512], ps[:, :],
                        1.0 / 64, bqk_sb[:, 8:9], MULT, ADD)
                else:
                    for cb in range(CB):
                        nc.tensor.matmul(
                            ps[:, :],
                            wqkq_sb[0][:, cb * 128:(cb + 1) * 128],
                            x_sb[:, cb * N + nch * 512: cb * N + nch * 512 + 512],
                            start=(cb == 0), stop=(cb == CB - 1),
                        )
                    nc.vector.tensor_scalar(
                        qkq_sb[0][:, nch * 512:(nch + 1) * 512], ps[:, :],
                        bqk_sb[:, 0:1], None, ADD)

            for kq in range(2):
                for nch in range(2):
                    emit_B_direct(kq, nch)
            for tb in range(2):
                emit_A_direct(tb, 0)

            # ================= main attention loop =================
            def qk_mm(hp, ic, kb, hh, s_ps):
                r0, r1 = hh * 64, hh * 64 + 64
                nc.tensor.matmul(
                    s_ps[:, hh * 512:(hh + 1) * 512],
                    qkk_sb[hp][r0:r1, kb * 128:(kb + 1) * 128],
                    qkq_sb[hp][r0:r1, ic * 512:(ic + 1) * 512],
                    start=True, stop=True,
                )

            def emit_exp(s_ps):
                p_t = pool_pt.tile([128, N], BF16, tag="pt", name="pt")
                nc.scalar.activation(p_t[:, :], s_ps[:, :], EXP, scale=float(SCALE))
                return p_t

            def pv_mm(hp, kb, hh, p_t, o_ps):
                h = 2 * hp + hh
                nc.tensor.matmul(
                    o_ps[hh][:, :],
                    V_sb[kb][:, h * VW:(h + 1) * VW],
                    p_t[:, hh * 512:(hh + 1) * 512],
                    start=(kb == 0), stop=(kb == TB - 1),
                )

            def emit_norm(hp, hh, ic, o_cp):
                # normalize one query-half as soon as its pass is evacuated
                c0, c1 = ic * 512, (ic + 1) * 512
                s128 = pool_norm.tile([128, 4], F32, tag=f"s128_{hh}",
                                      name="s128", bufs=3)
                nc.sync.dma_start(s128[:, :], o_cp[hh][64:65, c0:c1])
                r128 = pool_norm.tile([128, 4], F32, tag="r128", bufs=3)
                nc.vector.reciprocal(r128[:, :], s128[:, :])
                r0t = pool_norm.tile([1, 512], F32, tag="r0", bufs=3)
                nc.sync.dma_start(r0t[0:1, :], r128[:, :])
                r_rep = pool_norm.tile([64, 512], F32, tag="rrep", bufs=3)
                nc.gpsimd.partition_broadcast(r_rep[:, :], r0t[0:1, :])
                if hh == 0:
                    nc.vector.tensor_mul(
                        A_sb[hp][0:64, c0:c1], o_cp[hh][0:64, c0:c1], r_rep[:, :])
                else:
                    a_tmp = pool_norm.tile([64, 512], BF16, tag="atmp", bufs=3)
                    nc.vector.tensor_mul(a_tmp[:, :], o_cp[hh][0:64, c0:c1],
                                         r_rep[:, :])
                    nc.sync.dma_start(A_sb[hp][64:128, c0:c1], a_tmp[:, :])

            for hp in range(8):
                o_cp = [pool_ocp.tile([VW, N], F32, tag=f"ocp{hh}",
                                      name=f"ocp{hh}", bufs=2) for hh in range(2)]
                for ic in range(2):
                    o_ps = [ps_O.tile([VW, 512], F32, tag=f"O{hh}", name=f"O{hh}") for hh in range(2)]
                    s0 = ps_S.tile([128, N], F32, tag="S", name="S")
                    qk_mm(hp, ic, 0, 0, s0)
                    qk_mm(hp, ic, 0, 1, s0)
                    pt_q = [emit_exp(s0)]
                    for kb in range(TB):
                        # head-1 PV of the previous unit sits between this
                        # unit's two QK matmuls: all four are bf16 single MMs,
                        # so the PE weight buffers strictly alternate and every
                        # LDWEIGHTS hides under the previous 512-col stream
                        if kb + 1 < TB:
                            s_n = ps_S.tile([128, N], F32, tag="S", name="S")
                            qk_mm(hp, ic, kb + 1, 0, s_n)
                            if kb >= 1:
                                pv_mm(hp, kb - 1, 1, pt_q[kb - 1], o_ps)
                            qk_mm(hp, ic, kb + 1, 1, s_n)
                            pt_q.append(emit_exp(s_n))
                        elif kb >= 1:
                            pv_mm(hp, kb - 1, 1, pt_q[kb - 1], o_ps)
                        if hp == 0 and ic == 0:
                            if kb + 2 < TB:
                                emit_A_direct(kb + 2, 0)
                        elif hp == 7:
                            emit_D1b_chunk(kb, ic)
                        pv_mm(hp, kb, 0, pt_q[kb], o_ps)
                        if not (hp == 0 and ic == 0) and hp != 7:
                            drain1(4)
                    pv_mm(hp, TB - 1, 1, pt_q[TB - 1], o_ps)
                    for hh in range(2):
                        nc.vector.tensor_copy(
                            o_cp[hh][:, ic * 512:(ic + 1) * 512], o_ps[hh][:, :])
                    emit_norm(hp, 0, ic, o_cp)
                    emit_norm(hp, 1, ic, o_cp)

            drain1(len(fill_q))

            # ================= output projection tail =================
            for cb in range(CB):
                for nch in range(2):
                    ps = ps_proj.tile([128, 512], F32, tag="pp", name="pp")
                    for hb in range(6, 8):
                        nc.tensor.matmul(
                            ps[:, :],
                            wo_sb[:, hb * C + cb * 128: hb * C + cb * 128 + 128],
                            A_sb[hb][:, nch * 512:(nch + 1) * 512],
                            start=(hb == 6), stop=(hb == 7),
                        )
                    g = cb * 2 + nch
                    o_t = pool_out.tile([128, 512], F32, tag="ot")
                    nc.vector.tensor_add(
                        o_t[:, :], ps[:, :], x_sb[:, g * 512:(g + 1) * 512])
                    eng = nc.sync if nch == 0 else nc.scalar
                    eng.dma_start(
                        outT[cb * 128:(cb + 1) * 128, nch * 512:(nch + 1) * 512],
                        o_t[:, :],
                    )
    nc.compile()
    return nc


def _get_nc():
    if "nc" not in _COMPILED:
        _COMPILED["nc"] = _build()
    return _COMPILED["nc"]


def _swizzle(mT):
    """[C, W] -> [128, CB*W]: row cb*128+p goes to partition p, block cb."""
    c, w = mT.shape
    return np.ascontiguousarray(
        mT.reshape(CB, 128, w).transpose(1, 0, 2).reshape(128, CB * w))


def _run(x, in_proj_weight, in_proj_bias, out_proj_weight, out_proj_bias,
         trace=False):
    import ml_dtypes
    from concourse.bass_utils import run_bass_kernel_spmd

    BF = ml_dtypes.bfloat16
    nc = _get_nc()
    x = np.asarray(x, dtype=np.float32)
    w_in = np.asarray(in_proj_weight, dtype=np.float32)
    b_in = np.asarray(in_proj_bias, dtype=np.float32)
    w_out = np.asarray(out_proj_weight, dtype=np.float32)
    b_out = np.asarray(out_proj_bias, dtype=np.float32)

    F8 = ml_dtypes.float8_e4m3
    qT = np.ascontiguousarray(w_in[0:C].T)            # [C, C] q features
    kT = np.ascontiguousarray(w_in[C:2 * C].T)        # [C, C] k features
    wqk8 = np.empty((8 * 128, CB * 128), dtype=np.float32)
    wqkq = np.empty((8 * 128, CB * 128), dtype=np.float32)
    for hp in range(8):
        for cb in range(CB):
            t, j = cb // 2, cb % 2
            base = t * 256 + j * 128
            wqk8[hp * 128:(hp + 1) * 128, base:base + 128] = \
                kT[cb * 128:(cb + 1) * 128, hp * 128:(hp + 1) * 128] * 64
            wqkq[hp * 128:(hp + 1) * 128, cb * 128:(cb + 1) * 128] = \
                qT[cb * 128:(cb + 1) * 128, hp * 128:(hp + 1) * 128]

    shared = {
        "wqk8_d": wqk8.astype(F8),
        "wqkq_d": wqkq.astype(BF),
        "wv0_d": _swizzle(np.ascontiguousarray(w_in[2 * C:3 * C].T[:, 0:512])).astype(BF),
        "wv1_d": _swizzle(np.ascontiguousarray(w_in[2 * C:3 * C].T[:, 512:1024])).astype(BF),
        "wo_d": _swizzle(w_out.T).astype(BF),
        "bqk_d": np.ascontiguousarray(b_in[0:2 * C].reshape(16, 128).T),
        "bv_d": np.ascontiguousarray(b_in[2 * C:3 * C])[None, :],
        "bo_d": np.ascontiguousarray(b_out.reshape(8, 128).T),
    }
    in_maps = []
    for c in range(NCORES):
        m = dict(shared)
        xs = _swizzle(np.ascontiguousarray(x[c].T))
        m["x_d"] = xs.astype(BF)
        m["x8_d"] = xs.astype(F8)
        in_maps.append(m)

    res = run_bass_kernel_spmd(nc, in_maps, core_ids=list(range(NCORES)),
                               trace=trace)
    out = np.stack([
        np.ascontiguousarray(res.results[c]["outT"].T) for c in range(NCORES)
    ]).astype(np.float32)
    return out, res


def kernel(x, in_proj_weight, in_proj_bias, out_proj_weight, out_proj_bias):
    out, _ = _run(x, in_proj_weight, in_proj_bias, out_proj_weight,
                  out_proj_bias)
    return out


# revision 25
# speedup vs baseline: 1.3112x; 1.3112x over previous
"""Multi-head attention (B=8, N=1024, C=1024, H=16) on 8 TRN2 NeuronCores.

Strategy: pure data parallelism — each core computes one batch element with
replicated weights (no collectives). All matmul operands are bf16 (fast
weight loads + half DMA); PSUM accumulation stays fp32.

The kernel is emitted as ONE software-pipelined stream so the scalar engine
(the only engine with exp) is busy wall-to-wall instead of only during a
trailing attention phase:

  per head-pair hp (2 heads), per query-half ic, per key-block kb:
    QK: S^T[key128, q512] for both heads via row-group-paired matmuls
    ACT: P = exp(S*scale)  (scalar engine — the critical resource)
    PV: O^T[65, q512] += V_hat^T @ P^T   (row 64 = softmax sums via ones col)
  "filler" work (QKV projections for later pairs, V projection, first half
  of the output projection) is interleaved into the tensor engine's slack.

Layouts (host-prepped, partition-major so every matmul contracts on
partitions): x/wv/wo as [128, 8*1024] swizzled blocks, wqk packed per
head-pair [128, 8*(128k|128q)]. Output is outT [C, N] fp32, host transposes.
"""
import numpy as np

B, N, C = 8, 1024, 1024
H = 16
HD = C // H               # 64
SCALE = HD ** (-0.5)
NCORES = 8
CB = 8                    # contraction blocks (C / 128)
TB = 8                    # token blocks (N / 128)
VW = 65                   # per-head V width (64 feats + ones col)

_COMPILED = {}


def _build():
    import concourse.bass as bass
    import concourse.tile as tile
    from concourse import bacc, mybir

    F32 = mybir.dt.float32
    BF16 = mybir.dt.bfloat16
    EXP = mybir.ActivationFunctionType.Exp
    ADD = mybir.AluOpType.add

    nc = bacc.Bacc("TRN2", target_bir_lowering=False, debug=False)

    x_d = nc.dram_tensor("x_d", [128, CB * N], BF16, kind="ExternalInput").ap()
    x8_d = nc.dram_tensor("x8_d", [128, CB * N], mybir.dt.float8e4,
                          kind="ExternalInput").ap()
    wqk8_d = nc.dram_tensor("wqk8_d", [8 * 128, CB * 128], mybir.dt.float8e4,
                            kind="ExternalInput").ap()
    wqkq_d = nc.dram_tensor("wqkq_d", [8 * 128, CB * 128], BF16,
                            kind="ExternalInput").ap()
    wv_d = [nc.dram_tensor(f"wv{vc}_d", [128, CB * 512], BF16,
                           kind="ExternalInput").ap() for vc in range(2)]
    wo_d = nc.dram_tensor("wo_d", [128, CB * C], BF16, kind="ExternalInput").ap()
    bqk_d = nc.dram_tensor("bqk_d", [128, 16], F32, kind="ExternalInput").ap()
    bv_d = nc.dram_tensor("bv_d", [1, C], F32, kind="ExternalInput").ap()
    bo_d = nc.dram_tensor("bo_d", [128, 8], F32, kind="ExternalInput").ap()
    outT = nc.dram_tensor("outT", [C, N], F32, kind="ExternalOutput").ap()

    with nc.allow_low_precision(reason="attention: 2e-2 tolerance, bf16 "
                                "normalization chain is well within budget"), \
         tile.TileContext(nc) as tc:
        with tc.tile_pool(name="misc", bufs=1) as pool_misc, \
             tc.tile_pool(name="w", bufs=1) as pool_w, \
             tc.tile_pool(name="qk", bufs=1) as pool_qk, \
             tc.tile_pool(name="V", bufs=1) as pool_V, \
             tc.tile_pool(name="pt", bufs=3) as pool_pt, \
             tc.tile_pool(name="ocp", bufs=1) as pool_ocp, \
             tc.tile_pool(name="opart", bufs=1) as pool_opart, \
             tc.tile_pool(name="norm", bufs=2) as pool_norm, \
             tc.tile_pool(name="outp", bufs=2) as pool_out, \
             tc.tile_pool(name="ps_S", bufs=2, space="PSUM") as ps_S, \
             tc.tile_pool(name="ps_O", bufs=1, space="PSUM") as ps_O, \
             tc.tile_pool(name="ps_proj", bufs=2, space="PSUM") as ps_proj:

            # ---- small constants ----
            bqk_sb = pool_misc.tile([128, 16], F32, tag="bqk")
            bv_sb = pool_misc.tile([1, C], F32, tag="bv")
            bo_sb = pool_misc.tile([128, 8], F32, tag="bo")
            nc.sync.dma_start(bqk_sb[:, :], bqk_d)
            nc.sync.dma_start(bv_sb[:, :], bv_d)
            nc.sync.dma_start(bo_sb[:, :], bo_d)
            bv_rep = pool_misc.tile([128, C], F32, tag="bvrep")
            nc.gpsimd.partition_broadcast(bv_rep[:, :], bv_sb[0:1, :])

            # ---- big SBUF tensors ----
            x_sb = pool_w.tile([128, CB * N], BF16, tag="x")
            F8 = mybir.dt.float8e4
            x8_sb = pool_w.tile([128, CB * N], F8, tag="x8")
            wqk8_sb = [pool_w.tile([128, CB * 128], F8, tag=f"wqk8{hp}", name=f"wqk8{hp}")
                       for hp in range(8)]
            wqkq_sb = [pool_w.tile([128, CB * 128], BF16, tag=f"wqkq{hp}", name=f"wqkq{hp}")
                       for hp in range(8)]
            wv_sb = [pool_w.tile([128, CB * 512], BF16, tag=f"wv{vc}", name=f"wv{vc}")
                     for vc in range(2)]
            wo_sb = pool_w.tile([128, CB * C], BF16, tag="wo")
            qkk_sb = [pool_qk.tile([128, N], BF16, tag=f"qkk{hp}", name=f"qkk{hp}") for hp in range(8)]
            qkq_sb = [pool_qk.tile([128, N], BF16, tag=f"qkq{hp}", name=f"qkq{hp}") for hp in range(8)]
            A_sb = qkq_sb  # normalized attention outputs reuse the q tiles
            V_sb = [pool_V.tile([128, H * VW], BF16, tag=f"V{tb}", name=f"V{tb}") for tb in range(TB)]
            o_part = [pool_opart.tile([128, 512], BF16, tag=f"op{g}", name=f"op{g}")
                      for g in range(16)]

            # ---- PE warm-up: ~9us of dummy matmuls so the HAM clock gate
            # opens during the input DMA and projections start at 2.4 GHz
            warm_sb = pool_misc.tile([128, 512], BF16, tag="warm")
            nc.vector.memset(warm_sb[:, :], 0.0)
            for w in range(40):
                ps = ps_proj.tile([128, 512], F32, tag="pp", name="pp")
                nc.tensor.matmul(ps[:, :], warm_sb[:, 0:128], warm_sb[:, :],
                                 start=True, stop=True)

            # ---- input DMA: x/wv/wo on sync, wqkp on scalar (parallel
            # queues; ACT is idle early). V ones columns via memset.
            for tb in range(TB):
                nc.vector.memset(V_sb[tb][:, 64::VW], 1.0)
            nc.scalar.dma_start(x8_sb[:, :], x8_d)
            nc.sync.dma_start(x_sb[:, :], x_d)
            nc.scalar.dma_start(wqk8_sb[0][:, :], wqk8_d[0:128, :])
            nc.scalar.dma_start(wqkq_sb[0][:, :], wqkq_d[0:128, :])
            for hp in range(1, 8):
                nc.scalar.dma_start(wqk8_sb[hp][:, :],
                                    wqk8_d[hp * 128:(hp + 1) * 128, :])
                nc.scalar.dma_start(wqkq_sb[hp][:, :],
                                    wqkq_d[hp * 128:(hp + 1) * 128, :])
            for vc in range(2):
                nc.sync.dma_start(wv_sb[vc][:, :], wv_d[vc])
            nc.sync.dma_start(wo_sb[:, :], wo_d)

            # ================= emission helpers =================
            MULT = mybir.AluOpType.mult
            DR = mybir.MatmulPerfMode.DoubleRow

            def emit_B_chunk(hp, kq, nch, half, state):
                # kq=0: k features via fp8 DoubleRow (256-row contractions);
                # kq=1: q features via bf16 (accuracy headroom is tighter on q+k
                # jointly, so only one side is quantized to fp8)
                if half == 0:
                    state["ps"] = ps_proj.tile([128, 512], F32, tag="pp", name="pp")
                ps = state["ps"]
                if kq == 0:
                    for t in (2 * half, 2 * half + 1):
                        lhsT = wqk8_sb[hp][:, t * 256:(t + 1) * 256].rearrange(
                            "p (j m) -> p j m", j=2)
                        rhs = x8_sb[:, 2 * t * N:2 * (t + 1) * N].rearrange(
                            "p (j n) -> p j n", j=2)[:, :, nch * 512:(nch + 1) * 512]
                        nc.tensor.matmul(ps[:, :], lhsT, rhs,
                                         start=(t == 0), stop=(t == 3), perf_mode=DR)
                    if half == 1:
                        nc.vector.tensor_scalar(
                            qkk_sb[hp][:, nch * 512:(nch + 1) * 512], ps[:, :],
                            1.0 / 64, bqk_sb[:, 8 + hp:9 + hp], MULT, ADD)
                else:
                    for cb in range(4 * half, 4 * half + 4):
                        nc.tensor.matmul(
                            ps[:, :],
                            wqkq_sb[hp][:, cb * 128:(cb + 1) * 128],
                            x_sb[:, cb * N + nch * 512: cb * N + nch * 512 + 512],
                            start=(cb == 0), stop=(cb == CB - 1),
                        )
                    if half == 1:
                        nc.vector.tensor_scalar(
                            qkq_sb[hp][:, nch * 512:(nch + 1) * 512], ps[:, :],
                            bqk_sb[:, hp:hp + 1], None, ADD)

            def emit_A_chunk(tb, vc, half, state):
                if half == 0:
                    state["ps"] = ps_proj.tile([128, 512], F32, tag="pp", name="pp")
                ps = state["ps"]
                for cb in range(4 * half, 4 * half + 4):
                    nc.tensor.matmul(
                        ps[:, :],
                        x_sb[:, cb * N + tb * 128: cb * N + tb * 128 + 128],
                        wv_sb[vc][:, cb * 512:(cb + 1) * 512],
                        start=(cb == 0), stop=(cb == CB - 1),
                    )
                if half == 1:
                    dst = V_sb[tb][:, vc * 8 * VW:(vc + 1) * 8 * VW]
                    dst3 = dst.rearrange("p (h d) -> p h d", h=8)[:, :, 0:64]
                    src3 = ps[:, :].rearrange("p (h d) -> p h d", h=8)
                    bv3 = bv_rep[:, vc * 512:(vc + 1) * 512].rearrange(
                        "p (h d) -> p h d", h=8)
                    nc.vector.tensor_add(dst3, src3, bv3)

            def emit_D1_chunk(cb, nch, state):
                ps = ps_proj.tile([128, 512], F32, tag="pp", name="pp")
                for hb in range(4):
                    nc.tensor.matmul(
                        ps[:, :],
                        wo_sb[:, hb * C + cb * 128: hb * C + cb * 128 + 128],
                        A_sb[hb][:, nch * 512:(nch + 1) * 512],
                        start=(hb == 0), stop=(hb == 3),
                    )
                nc.vector.tensor_scalar(
                    o_part[cb * 2 + nch][:, :], ps[:, :], bo_sb[:, cb:cb + 1],
                    None, ADD)

            def emit_D1b_chunk(cb, nch):
                # hb 4-5 partials; result = o_part(bias+hb0-3) + these, stored
                # into dead x_sb space so the tail only needs hb 6-7
                ps = ps_proj.tile([128, 512], F32, tag="pp", name="pp")
                for hb in range(4, 6):
                    nc.tensor.matmul(
                        ps[:, :],
                        wo_sb[:, hb * C + cb * 128: hb * C + cb * 128 + 128],
                        A_sb[hb][:, nch * 512:(nch + 1) * 512],
                        start=(hb == 4), stop=(hb == 5),
                    )
                g = cb * 2 + nch
                nc.vector.tensor_add(
                    x_sb[:, g * 512:(g + 1) * 512], ps[:, :], o_part[g][:, :])

            fill_q = []

            def add_B_group(hp, kq, nch):
                st = {}
                if kq == 0:
                    def mk(t):
                        def emit():
                            if t == 0:
                                st["ps"] = ps_proj.tile([128, 512], F32,
                                                        tag="pp", name="pp")
                            ps = st["ps"]
                            lhsT = wqk8_sb[hp][:, t * 256:(t + 1) * 256].rearrange(
                                "p (j m) -> p j m", j=2)
                            rhs = x8_sb[:, 2 * t * N:2 * (t + 1) * N].rearrange(
                                "p (j n) -> p j n", j=2)[:, :, nch * 512:(nch + 1) * 512]
                            nc.tensor.matmul(ps[:, :], lhsT, rhs,
                                             start=(t == 0), stop=(t == 3),
                                             perf_mode=DR)
                            if t == 3:
                                nc.vector.tensor_scalar(
                                    qkk_sb[hp][:, nch * 512:(nch + 1) * 512],
                                    ps[:, :], 1.0 / 64,
                                    bqk_sb[:, 8 + hp:9 + hp], MULT, ADD)
                        return emit
                    for t in range(4):
                        fill_q.append(mk(t))
                else:
                    def mk(cb):
                        def emit():
                            if cb == 0:
                                st["ps"] = ps_proj.tile([128, 512], F32,
                                                        tag="pp", name="pp")
                            ps = st["ps"]
                            nc.tensor.matmul(
                                ps[:, :],
                                wqkq_sb[hp][:, cb * 128:(cb + 1) * 128],
                                x_sb[:, cb * N + nch * 512: cb * N + nch * 512 + 512],
                                start=(cb == 0), stop=(cb == CB - 1),
                            )
                            if cb == CB - 1:
                                nc.vector.tensor_scalar(
                                    qkq_sb[hp][:, nch * 512:(nch + 1) * 512],
                                    ps[:, :], bqk_sb[:, hp:hp + 1], None, ADD)
                        return emit
                    for cb in range(CB):
                        fill_q.append(mk(cb))

            def add_A_group(tb, vc):
                st = {}
                def mk(cb):
                    def emit():
                        if cb == 0:
                            st["ps"] = ps_proj.tile([128, 512], F32,
                                                    tag="pp", name="pp")
                        ps = st["ps"]
                        nc.tensor.matmul(
                            ps[:, :],
                            x_sb[:, cb * N + tb * 128: cb * N + tb * 128 + 128],
                            wv_sb[vc][:, cb * 512:(cb + 1) * 512],
                            start=(cb == 0), stop=(cb == CB - 1),
                        )
                        if cb == CB - 1:
                            dst = V_sb[tb][:, vc * 8 * VW:(vc + 1) * 8 * VW]
                            dst3 = dst.rearrange("p (h d) -> p h d", h=8)[:, :, 0:64]
                            src3 = ps[:, :].rearrange("p (h d) -> p h d", h=8)
                            bv3 = bv_rep[:, vc * 512:(vc + 1) * 512].rearrange(
                                "p (h d) -> p h d", h=8)
                            nc.vector.tensor_add(dst3, src3, bv3)
                    return emit
                for cb in range(CB):
                    fill_q.append(mk(cb))

            def add_D1_group(cb, nch):
                st = {}
                def mk(hb):
                    def emit():
                        if hb == 0:
                            st["ps"] = ps_proj.tile([128, 512], F32,
                                                    tag="pp", name="pp")
                        ps = st["ps"]
                        nc.tensor.matmul(
                            ps[:, :],
                            wo_sb[:, hb * C + cb * 128: hb * C + cb * 128 + 128],
                            A_sb[hb][:, nch * 512:(nch + 1) * 512],
                            start=(hb == 0), stop=(hb == 3),
                        )
                        if hb == 3:
                            nc.vector.tensor_scalar(
                                o_part[cb * 2 + nch][:, :], ps[:, :],
                                bo_sb[:, cb:cb + 1], None, ADD)
                    return emit
                for hb in range(4):
                    fill_q.append(mk(hb))

            for hp in (1, 2, 3, 4):
                for kq in range(2):
                    for nch in range(2):
                        add_B_group(hp, kq, nch)
            for tb in range(TB):
                add_A_group(tb, 1)
            for hp in (5, 6, 7):
                for kq in range(2):
                    for nch in range(2):
                        add_B_group(hp, kq, nch)
            for cb in range(CB):
                for nch in range(2):
                    add_D1_group(cb, nch)

            fill_i = 0

            def drain1(k=1):
                nonlocal fill_i
                for _ in range(k):
                    if fill_i < len(fill_q):
                        fill_q[fill_i]()
                        fill_i += 1

            # ---- pair 0 prologue: its qk projection + first V blocks ----
            def emit_A_direct(tb, vc):
                ps = ps_proj.tile([128, 512], F32, tag="pp", name="pp")
                for cb in range(CB):
                    nc.tensor.matmul(
                        ps[:, :],
                        x_sb[:, cb * N + tb * 128: cb * N + tb * 128 + 128],
                        wv_sb[vc][:, cb * 512:(cb + 1) * 512],
                        start=(cb == 0), stop=(cb == CB - 1),
                    )
                dst = V_sb[tb][:, vc * 8 * VW:(vc + 1) * 8 * VW]
                dst3 = dst.rearrange("p (h d) -> p h d", h=8)[:, :, 0:64]
                src3 = ps[:, :].rearrange("p (h d) -> p h d", h=8)
                bv3 = bv_rep[:, vc * 512:(vc + 1) * 512].rearrange(
                    "p (h d) -> p h d", h=8)
                nc.vector.tensor_add(dst3, src3, bv3)

            def emit_B_direct(kq, nch):
                ps = ps_proj.tile([128, 512], F32, tag="pp", name="pp")
                if kq == 0:
                    for t in range(4):
                        lhsT = wqk8_sb[0][:, t * 256:(t + 1) * 256].rearrange(
                            "p (j m) -> p j m", j=2)
                        rhs = x8_sb[:, 2 * t * N:2 * (t + 1) * N].rearrange(
                            "p (j n) -> p j n", j=2)[:, :, nch * 512:(nch + 1) * 512]
                        nc.tensor.matmul(ps[:, :], lhsT, rhs,
                                         start=(t == 0), stop=(t == 3),
                                         perf_mode=DR)
                    nc.vector.tensor_scalar(
                        qkk_sb[0][:, nch * 512:(nch + 1) * 512], ps[:, :],
                        1.0 / 64, bqk_sb[:, 8:9], MULT, ADD)
                else:
                    for cb in range(CB):
                        nc.tensor.matmul(
                            ps[:, :],
                            wqkq_sb[0][:, cb * 128:(cb + 1) * 128],
                            x_sb[:, cb * N + nch * 512: cb * N + nch * 512 + 512],
                            start=(cb == 0), stop=(cb == CB - 1),
                        )
                    nc.vector.tensor_scalar(
                        qkq_sb[0][:, nch * 512:(nch + 1) * 512], ps[:, :],
                        bqk_sb[:, 0:1], None, ADD)

            for kq in range(2):
                for nch in range(2):
                    emit_B_direct(kq, nch)
            for tb in range(2):
                emit_A_direct(tb, 0)

            # ================= main attention loop =================
            def qk_mm(hp, ic, kb, hh, s_ps):
                r0, r1 = hh * 64, hh * 64 + 64
                nc.tensor.matmul(
                    s_ps[:, hh * 512:(hh + 1) * 512],
                    qkk_sb[hp][r0:r1, kb * 128:(kb + 1) * 128],
                    qkq_sb[hp][r0:r1, ic * 512:(ic + 1) * 512],
                    start=True, stop=True,
                )

            def emit_exp(s_ps):
                p_t = pool_pt.tile([128, N], BF16, tag="pt", name="pt")
                nc.scalar.activation(p_t[:, :], s_ps[:, :], EXP, scale=float(SCALE))
                return p_t

            def pv_mm(hp, kb, hh, p_t, o_ps):
                h = 2 * hp + hh
                nc.tensor.matmul(
                    o_ps[hh][:, :],
                    V_sb[kb][:, h * VW:(h + 1) * VW],
                    p_t[:, hh * 512:(hh + 1) * 512],
                    start=(kb == 0), stop=(kb == TB - 1),
                )

            def emit_norm(hp, hh, ic, o_cp):
                # normalize one query-half as soon as its pass is evacuated
                c0, c1 = ic * 512, (ic + 1) * 512
                s128 = pool_norm.tile([128, 4], F32, tag=f"s128_{hh}",
                                      name="s128", bufs=3)
                nc.sync.dma_start(s128[:, :], o_cp[hh][64:65, c0:c1])
                r128 = pool_norm.tile([128, 4], F32, tag="r128", bufs=3)
                nc.vector.reciprocal(r128[:, :], s128[:, :])
                r0t = pool_norm.tile([1, 512], F32, tag="r0", bufs=3)
                nc.sync.dma_start(r0t[0:1, :], r128[:, :])
                r_rep = pool_norm.tile([64, 512], F32, tag="rrep", bufs=3)
                nc.gpsimd.partition_broadcast(r_rep[:, :], r0t[0:1, :])
                if hh == 0:
                    nc.vector.tensor_mul(
                        A_sb[hp][0:64, c0:c1], o_cp[hh][0:64, c0:c1], r_rep[:, :])
                else:
                    a_tmp = pool_norm.tile([64, 512], BF16, tag="atmp", bufs=3)
                    nc.vector.tensor_mul(a_tmp[:, :], o_cp[hh][0:64, c0:c1],
                                         r_rep[:, :])
                    nc.sync.dma_start(A_sb[hp][64:128, c0:c1], a_tmp[:, :])

            for hp in range(8):
                o_cp = [pool_ocp.tile([VW, N], F32, tag=f"ocp{hh}",
                                      name=f"ocp{hh}", bufs=2) for hh in range(2)]
                for ic in range(2):
                    o_ps = [ps_O.tile([VW, 512], F32, tag=f"O{hh}", name=f"O{hh}") for hh in range(2)]
                    s0 = ps_S.tile([128, N], F32, tag="S", name="S")
                    qk_mm(hp, ic, 0, 0, s0)
                    qk_mm(hp, ic, 0, 1, s0)
                    pt_q = [emit_exp(s0)]
                    for kb in range(TB):
                        if kb + 1 < TB:
                            s_n = ps_S.tile([128, N], F32, tag="S", name="S")
                            qk_mm(hp, ic, kb + 1, 0, s_n)
                            qk_mm(hp, ic, kb + 1, 1, s_n)
                            pt_q.append(emit_exp(s_n))
                        if hp == 0 and ic == 0:
                            if kb + 2 < TB:
                                emit_A_direct(kb + 2, 0)
                        elif hp == 7:
                            emit_D1b_chunk(kb, ic)
                        pv_mm(hp, kb, 0, pt_q[kb], o_ps)
                        pv_mm(hp, kb, 1, pt_q[kb], o_ps)
                        if not (hp == 0 and ic == 0) and hp != 7:
                            drain1(4)
                    for hh in range(2):
                        nc.vector.tensor_copy(
                            o_cp[hh][:, ic * 512:(ic + 1) * 512], o_ps[hh][:, :])
                    emit_norm(hp, 0, ic, o_cp)
                    emit_norm(hp, 1, ic, o_cp)

            drain1(len(fill_q))

            # ================= output projection tail =================
            for cb in range(CB):
                for nch in range(2):
                    ps = ps_proj.tile([128, 512], F32, tag="pp", name="pp")
                    for hb in range(6, 8):
                        nc.tensor.matmul(
                            ps[:, :],
                            wo_sb[:, hb * C + cb * 128: hb * C + cb * 128 + 128],
                            A_sb[hb][:, nch * 512:(nch + 1) * 512],
                            start=(hb == 6), stop=(hb == 7),
                        )
                    g = cb * 2 + nch
                    o_t = pool_out.tile([128, 512], F32, tag="ot")
                    nc.vector.tensor_add(
                        o_t[:, :], ps[:, :], x_sb[:, g * 512:(g + 1) * 512])
                    eng = nc.sync if nch == 0 else nc.scalar
                    eng.dma_start(
                        outT[cb * 128:(cb + 1) * 128, nch * 512:(nch + 1) * 512],
                        o_t[:, :],
                    )
    nc.compile()
    return nc


def _get_nc():
    if "nc" not in _COMPILED:
        _COMPILED["nc"] = _build()
    return _COMPILED["nc"]


def _swizzle(mT):
    """[C, W] -> [128, CB*W]: row cb*128+p goes to partition p, block cb."""
    c, w = mT.shape
    return np.ascontiguousarray(
        mT.reshape(CB, 128, w).transpose(1, 0, 2).reshape(128, CB * w))


def _run(x, in_proj_weight, in_proj_bias, out_proj_weight, out_proj_bias,
         trace=False):
    import ml_dtypes
    from concourse.bass_utils import run_bass_kernel_spmd

    BF = ml_dtypes.bfloat16
    nc = _get_nc()
    x = np.asarray(x, dtype=np.float32)
    w_in = np.asarray(in_proj_weight, dtype=np.float32)
    b_in = np.asarray(in_proj_bias, dtype=np.float32)
    w_out = np.asarray(out_proj_weight, dtype=np.float32)
    b_out = np.asarray(out_proj_bias, dtype=np.float32)

    F8 = ml_dtypes.float8_e4m3
    qT = np.ascontiguousarray(w_in[0:C].T)            # [C, C] q features
    kT = np.ascontiguousarray(w_in[C:2 * C].T)        # [C, C] k features
    wqk8 = np.empty((8 * 128, CB * 128), dtype=np.float32)
    wqkq = np.empty((8 * 128, CB * 128), dtype=np.float32)
    for hp in range(8):
        for cb in range(CB):
            t, j = cb // 2, cb % 2
            base = t * 256 + j * 128
            wqk8[hp * 128:(hp + 1) * 128, base:base + 128] = \
                kT[cb * 128:(cb + 1) * 128, hp * 128:(hp + 1) * 128] * 64
            wqkq[hp * 128:(hp + 1) * 128, cb * 128:(cb + 1) * 128] = \
                qT[cb * 128:(cb + 1) * 128, hp * 128:(hp + 1) * 128]

    shared = {
        "wqk8_d": wqk8.astype(F8),
        "wqkq_d": wqkq.astype(BF),
        "wv0_d": _swizzle(np.ascontiguousarray(w_in[2 * C:3 * C].T[:, 0:512])).astype(BF),
        "wv1_d": _swizzle(np.ascontiguousarray(w_in[2 * C:3 * C].T[:, 512:1024])).astype(BF),
        "wo_d": _swizzle(w_out.T).astype(BF),
        "bqk_d": np.ascontiguousarray(b_in[0:2 * C].reshape(16, 128).T),
        "bv_d": np.ascontiguousarray(b_in[2 * C:3 * C])[None, :],
        "bo_d": np.ascontiguousarray(b_out.reshape(8, 128).T),
    }
    in_maps = []
    for c in range(NCORES):
        m = dict(shared)
        xs = _swizzle(np.ascontiguousarray(x[c].T))
        m["x_d"] = xs.astype(BF)
        m["x8_d"] = xs.astype(F8)
        in_maps.append(m)

    res = run_bass_kernel_spmd(nc, in_maps, core_ids=list(range(NCORES)),
                               trace=trace)
    out = np.stack([
        np.ascontiguousarray(res.results[c]["outT"].T) for c in range(NCORES)
    ]).astype(np.float32)
    return out, res


def kernel(x, in_proj_weight, in_proj_bias, out_proj_weight, out_proj_bias):
    out, _ = _run(x, in_proj_weight, in_proj_bias, out_proj_weight,
                  out_proj_bias)
    return out
